# revision 16
# baseline (speedup 1.0000x reference)
"""ContextBottleneck kernel for 8 TRN2 NeuronCores.

Data-parallel over the 16384 tokens (2048 tokens/core); the small weights are
replicated. Per core:
  LayerNorm stats (DVE bn_stats) -> rsqrt via quake-seed Newton (DVE)
  -> normalize+cast fp8e4 (ACT Identity, per-partition scale/bias)
  -> DMA-xbar transpose of fp8 PAIRS viewed as u16 (y -> y^T, d on partitions;
     the pair interleave is absorbed by DoubleRow's [K,2,*] operand layout)
  -> matmul1 (PE, fp8e4 DoubleRow, 2x bf16 rate; W_down scaled by 64)
  -> SiLU+b_down bias (ACT, from PSUM, scale=1/64) -> s^T in fp8e4 pair tiles
  -> matmul2 (PE, DoubleRow, s^T stationary x alpha*W_up in fp8e5 moving)
     + alpha*b_up via K=1 bf16 ones-row matmul
  -> residual: half the psum tiles via DVE STT out=(1-alpha)*h+psum, the other
     half via PE f32r identity matmul (adds (1-alpha)h in PSUM) + ACT copy
  -> store bf16.
gamma/beta are folded into W_down / b_down host-side; alpha into W_up / b_up.
"""

import numpy as np
import ml_dtypes

import concourse.bacc as bacc
import concourse.tile as tile
from concourse import mybir
from concourse.tile import add_dep_helper
from concourse.bass_utils import run_bass_kernel_spmd

AF = mybir.ActivationFunctionType
ALU = mybir.AluOpType
PM = mybir.MatmulPerfMode
BF16 = mybir.dt.bfloat16
F32 = mybir.dt.float32
F32R = mybir.dt.float32r
F8E4 = mybir.dt.float8e4
F8E5 = mybir.dt.float8e5
F16 = mybir.dt.float16
U16 = mybir.dt.uint16
I32 = mybir.dt.int32

D = 2048
DB = 512
N_CORES = 8
KD2 = D // 256   # 8 DoubleRow contraction chunks for matmul1
KB2 = DB // 256  # 2 DoubleRow chunks for matmul2
NCOL = D // 512  # 4 output column chunks
LN_EPS = 1e-5
ALPHA = 0.01
CD = 64.0        # W_down fp8 scale, undone in SiLU's input scale


def build_kernel(T, one_minus_alpha, act_func=None):
    act_func = AF.Silu if act_func is None else act_func
    nc = bacc.Bacc(
        "TRN2",
        target_bir_lowering=False,
        debug=False,
        enable_asserts=True,
        num_devices=N_CORES,
    )
    h_d = nc.dram_tensor("h", [T, D], F32R, kind="ExternalInput").ap()
    wd_d = nc.dram_tensor("wd", [128, KD2 * 2 * DB], F8E4, kind="ExternalInput").ap()
    wu_d = nc.dram_tensor("wu", [128, KB2 * 2 * D], F8E5, kind="ExternalInput").ap()
    b1_d = nc.dram_tensor("b1", [128, 4], F32, kind="ExternalInput").ap()
    bu_d = nc.dram_tensor("bu", [1, D], BF16, kind="ExternalInput").ap()
    id_d = nc.dram_tensor("ident", [128, 128], F32R, kind="ExternalInput").ap()
    o_d = nc.dram_tensor("o", [T, D], F16, kind="ExternalOutput").ap()

    n_groups = T // 512
    assert T % 512 == 0

    with tile.TileContext(nc) as tc:
        with (
            tc.tile_pool(name="singles", bufs=1) as singles,
            tc.tile_pool(name="hp", bufs=12) as h_pool,
            tc.tile_pool(name="yp", bufs=3) as y_pool,
            tc.tile_pool(name="ytp", bufs=3) as yt_pool,
            tc.tile_pool(name="sp", bufs=6) as s_pool,
            tc.tile_pool(name="resp", bufs=3) as res_pool,
            tc.tile_pool(name="stp", bufs=6) as st_pool,
            tc.tile_pool(name="zpp", bufs=2, space="PSUM") as zp_pool,
            tc.tile_pool(name="opp", bufs=3, space="PSUM") as op_pool,
        ):
            # weights ride the gpsimd (SWDGE) ring so they don't head-of-line
            # block the first activation loads on the SP ring
            wd_sb = singles.tile([128, KD2, 2, DB], F8E4)
            nc.gpsimd.dma_start(
                wd_sb[:].rearrange("p a b c -> p (a b c)"), wd_d[:]
            )
            wu_sb = singles.tile([128, KB2, 2, D], F8E5)
            nc.gpsimd.dma_start(
                wu_sb[:].rearrange("p a b c -> p (a b c)"), wu_d[:]
            )
            b1_sb = singles.tile([128, 4], F32)
            nc.gpsimd.dma_start(b1_sb[:], b1_d[:])
            bu_sb = singles.tile([1, D], BF16)
            nc.gpsimd.dma_start(bu_sb[:], bu_d[:])
            ones_sb = singles.tile([1, 128], BF16)
            nc.vector.memset(ones_sb[:], 1.0)
            # (1-alpha)*I for the PE residual add (f32, used as f32r)
            ident_sb = singles.tile([128, 128], F32R)
            nc.gpsimd.dma_start(ident_sb[:], id_d[:])

            def emit_rsqrt(mean_ap, var_ap, n):
                """rsig = rsqrt(var+eps), nms = -mean*rsig, each [128, n].
                Quake seed + 1 Newton round (rel err ~5e-4, damped by alpha)."""
                with tc.high_priority():
                    a = st_pool.tile([128, n], F32, tag=f"qa{n}")
                    nc.vector.tensor_scalar_add(a[:], var_ap, LN_EPS)
                    ya = st_pool.tile([128, n], F32, tag=f"qya{n}")
                    yb = st_pool.tile([128, n], F32, tag=f"qyb{n}")
                    t1 = st_pool.tile([128, n], F32, tag=f"qt1{n}")
                    t2 = st_pool.tile([128, n], F32, tag=f"qt2{n}")
                    nc.vector.tensor_scalar(
                        t1[:].bitcast(I32),
                        a[:].bitcast(I32),
                        1,
                        -1,
                        ALU.logical_shift_right,
                        ALU.bitwise_xor,
                    )
                    nc.vector.tensor_scalar(
                        ya[:].bitcast(I32),
                        t1[:].bitcast(I32),
                        0x5F3759E0,
                        None,
                        ALU.add,
                    )
                    cur, nxt = ya, yb
                    for _ in range(1):
                        nc.vector.tensor_mul(t1[:], cur[:], cur[:])
                        nc.vector.scalar_tensor_tensor(
                            t2[:], t1[:], -0.5, a[:], ALU.mult, ALU.mult
                        )
                        nc.vector.scalar_tensor_tensor(
                            nxt[:], t2[:], 1.5, cur[:], ALU.add, ALU.mult
                        )
                        cur, nxt = nxt, cur
                    rsig = cur  # [128, n]
                    nms = st_pool.tile([128, n], F32, tag=f"nms{n}")
                    nc.vector.scalar_tensor_tensor(
                        nms[:], mean_ap, -1.0, rsig[:], ALU.mult, ALU.mult
                    )
                return rsig, nms

            def emit_ln(g):
                """LayerNorm stage for group g: per-tile loads, stats, rsqrt,
                normalize+cast fp8, u16-pair transpose. Returns (h_tiles, yts)."""
                h_tiles = []
                yts = yt_pool.tile([128, KD2, 512], U16, tag="yts")
                yts8 = yts[:].bitcast(F8E4)  # [128, KD2, 1024]
                mvg = st_pool.tile([128, 4, 2], F32, tag="mvg")
                per_tile = g == 0  # group 0: no cross-tile rsqrt barrier
                for j in range(4):
                    ht = h_pool.tile([128, D], F32R, tag="ht")
                    row0 = (g * 4 + j) * 128
                    # alternate the two HWDGE rings (SP / ACT) for h loads
                    ring = nc.sync if j % 2 == 0 else nc.scalar
                    ring.dma_start(ht[:], h_d[row0 : row0 + 128, :])
                    h_tiles.append(ht)
                    st6 = st_pool.tile([128, 4, 6], F32, tag="st6")
                    for sub in range(4):
                        nc.vector.bn_stats(
                            st6[:, sub, :],
                            ht[:, sub * 512 : (sub + 1) * 512].bitcast(F32)
                        )
                    nc.vector.bn_aggr(mvg[:, j, :], st6[:])
                    if per_tile:
                        rsig_j, nms_j = emit_rsqrt(
                            mvg[:, j, 0:1], mvg[:, j, 1:2], 1
                        )
                        yt_ = y_pool.tile([128, D], F8E4, tag="yt_")
                        nc.scalar.activation(
                            yt_[:],
                            ht[:].bitcast(F32),
                            AF.Identity,
                            bias=nms_j[:, 0:1],
                            scale=rsig_j[:, 0:1],
                        )
                        nc.sync.dma_start_transpose(
                            yts[:, :, j * 128 : (j + 1) * 128],
                            yt_[:].bitcast(U16),
                        )
                if per_tile:
                    return h_tiles, yts8

                rsig, nms = emit_rsqrt(mvg[:, :, 0], mvg[:, :, 1], 4)

                for j in range(4):
                    yt_ = y_pool.tile([128, D], F8E4, tag="yt_")
                    nc.scalar.activation(
                        yt_[:],
                        h_tiles[j][:].bitcast(F32),
                        AF.Identity,
                        bias=nms[:, j : j + 1],
                        scale=rsig[:, j : j + 1],
                    )
                    nc.sync.dma_start_transpose(
                        yts[:, :, j * 128 : (j + 1) * 128],
                        yt_[:].bitcast(U16),
                    )
                return h_tiles, yts8

            def emit_compute(g, h_tiles, yts8):
                """matmul1 + SiLU + matmul2(+bias+residual) + drain + store."""
                sg_tiles = []
                for k in range(KB2):
                    sg_k = s_pool.tile([128, 2, 512], F8E4, tag=f"sg{k}", name=f"sg{k}")
                    sg_tiles.append(sg_k)
                for db in range(4):
                    zp = zp_pool.tile([128, 512], F32, tag="zp")
                    for k2 in range(KD2):
                        nc.tensor.matmul(
                            zp[:],
                            wd_sb[:, k2, :, db * 128 : (db + 1) * 128],
                            yts8[:, k2, :].rearrange("p (n i) -> p i n", i=2),
                            start=(k2 == 0),
                            stop=(k2 == KD2 - 1),
                            perf_mode=PM.DoubleRow,
                        )
                    nc.scalar.activation(
                        sg_tiles[db // 2][:, db % 2, :],
                        zp[:],
                        act_func,
                        bias=b1_sb[:, db : db + 1],
                        scale=1.0 / CD,
                    )

                for j in range(4):
                    # two [128, 2, 512] psum pair-tiles (2 banks each); one
                    # drain instruction per pair amortizes the per-op bubble
                    pairs = []
                    first_mm = None
                    for k in range(KB2):
                        for dcol in range(NCOL):
                            if k == 0 and dcol % 2 == 0:
                                pairs.append(
                                    op_pool.tile(
                                        [128, 2, 512], F32, tag="op_t", name="op_t"
                                    )
                                )
                            mm = nc.tensor.matmul(
                                pairs[dcol // 2][:, dcol % 2, :],
                                sg_tiles[k][:, :, j * 128 : (j + 1) * 128],
                                wu_sb[:, k, :, dcol * 512 : (dcol + 1) * 512],
                                start=(k == 0),
                                stop=False,
                                perf_mode=PM.DoubleRow,
                            )
                            if first_mm is None:
                                first_mm = mm
                    res = res_pool.tile([128, D], F16, tag="res")
                    for dcol in range(NCOL):
                        # alpha*b_up via a K=1 ones-row matmul, accumulated
                        # late.  The fake dep keeps constant-input matmuls
                        # from being hoisted ahead of the group (they would
                        # pin PSUM banks and stall the in-order PE queue).
                        act_drained = dcol >= 2
                        op_half = pairs[dcol // 2][:, dcol % 2, :]
                        bias_mm = nc.tensor.matmul(
                            op_half,
                            ones_sb[:, :],
                            bu_sb[:, dcol * 512 : (dcol + 1) * 512],
                            start=False,
                            stop=not act_drained,
                        )
                        add_dep_helper(
                            bias_mm.ins,
                            first_mm.ins,
                            sync=False,
                            reason="keep bias matmul with its group",
                        )
                        if act_drained:
                            # (1-alpha)*h added in PSUM by PE (f32r identity)
                            res_mm = nc.tensor.matmul(
                                op_half,
                                ident_sb[:],
                                h_tiles[j][:, dcol * 512 : (dcol + 1) * 512],
                                start=False,
                                stop=True,
                            )
                            add_dep_helper(
                                res_mm.ins,
                                first_mm.ins,
                                sync=False,
                                reason="keep resid matmul with its group",
                            )
                    # pair 0 (cols 0:1024): DVE STT residual drain
                    with tc.high_priority():
                        nc.vector.scalar_tensor_tensor(
                            res[:, 0:1024],
                            h_tiles[j][:, 0:1024].bitcast(F32),
                            one_minus_alpha,
                            pairs[0][:].rearrange("p a b -> p (a b)"),
                            ALU.mult,
                            ALU.add,
                        )
                    # pair 1 (cols 1024:2048): ACT copy (resid already in psum)
                    nc.scalar.copy(
                        res[:, 1024:2048],
                        pairs[1][:].rearrange("p a b -> p (a b)"),
                    )
                    row0 = (g * 4 + j) * 128
                    nc.gpsimd.dma_start(o_d[row0 : row0 + 128, :], res[:])

            # Software-pipelined emission: LN of group g+1 is emitted before
            # compute of group g so the per-engine FIFOs interleave the two
            # stages instead of serializing at group boundaries.
            stages = [emit_ln(g) for g in range(min(2, n_groups))]
            for g in range(n_groups):
                if g + 2 < n_groups:
                    stages.append(emit_ln(g + 2))
                emit_compute(g, *stages[g])

    nc.compile()
    return nc


def prep_host_inputs(hidden, ln_gamma, ln_beta, W_down, b_down, W_up, b_up, alpha):
    e4 = ml_dtypes.float8_e4m3
    e5 = ml_dtypes.float8_e5m2
    hidden = np.asarray(hidden, np.float32)
    gam = np.asarray(ln_gamma, np.float32)
    bet = np.asarray(ln_beta, np.float32)
    Wd = np.asarray(W_down, np.float32)
    bd = np.asarray(b_down, np.float32)
    Wu = np.asarray(W_up, np.float32)
    bu = np.asarray(b_up, np.float32)
    alpha = float(alpha)

    # fold gamma into W_down rows (scaled by CD for fp8 range); beta@W_down
    # into the bottleneck bias.  DoubleRow pair layout:
    #   wd8[p, k2, i, :] = CD * (gam*Wd)[256*k2 + 2p + i, :]
    wdg = (gam[:, None] * Wd) * CD  # [D, DB]
    wd8 = np.ascontiguousarray(
        wdg.reshape(KD2, 128, 2, DB).transpose(1, 0, 2, 3)  # p, k2, i, db
        .reshape(128, KD2 * 2 * DB)
    ).astype(e4)
    b1_h = np.ascontiguousarray(
        (bet @ Wd + bd).astype(np.float32).reshape(4, 128).T
    )  # [128, 4]
    # alpha*W_up in e5m2; DoubleRow pair layout over the Db contraction dim:
    #   wu8[p, k2, i, :] = alpha * Wu[128*(2*k2+i) + p, :]
    wua = alpha * Wu  # [DB, D]
    wu8 = np.ascontiguousarray(
        wua.reshape(KB2, 2, 128, D).transpose(2, 0, 1, 3)  # p, k2, i, n
        .reshape(128, KB2 * 2 * D)
    ).astype(e5)
    bu_h = np.ascontiguousarray((alpha * bu).astype(ml_dtypes.bfloat16).reshape(1, D))
    id_h = np.ascontiguousarray((1.0 - alpha) * np.eye(128, dtype=np.float32))
    flat = np.ascontiguousarray(hidden.reshape(-1, D))
    return flat, wd8, wu8, b1_h, bu_h, id_h, alpha


_cached = {}


def kernel(
    hidden,
    ln_gamma,
    ln_beta,
    W_down,
    b_down,
    W_up,
    b_up,
    alpha,
    layer_idx=None,
    **_unused,
):
    flat, wd8, wu8, b1_h, bu_h, id_h, alpha_f = prep_host_inputs(
        hidden, ln_gamma, ln_beta, W_down, b_down, W_up, b_up, alpha
    )
    T = flat.shape[0] // N_CORES
    key = (T, alpha_f)
    if key not in _cached:
        _cached[key] = build_kernel(T, 1.0 - alpha_f)
    nc = _cached[key]

    shards = flat.reshape(N_CORES, T, D)
    in_maps = [
        {
            "h": np.ascontiguousarray(shards[c]),
            "wd": wd8,
            "wu": wu8,
            "b1": b1_h,
            "bu": bu_h,
            "ident": id_h,
        }
        for c in range(N_CORES)
    ]
    res = run_bass_kernel_spmd(nc, in_maps, list(range(N_CORES)))
    global _last_results
    _last_results = res
    out = np.concatenate([r["o"] for r in res.results], axis=0)
    return out.reshape(np.asarray(hidden).shape).astype(np.float32)


_last_results = None
